# revision 8
# baseline (speedup 1.0000x reference)
"""MoE feed-forward (top-2 of 8 experts) Trainium2 Bass kernel.

Problem: nn_MixtureOfExpertsFeedForward_6734508720763
  x[4,1024,1024] tokens, router Wr[1024,8], experts W_in[8,1024,4096],
  W_out[8,4096,1024], top_k=2.

  ref:  logits = x@Wr + br ; probs = softmax(logits)
        top2 -> dispatch (0/1), combine (prob or 0)
        h = sum_e dispatch[n,e] * relu(x @ W_in[e] + b_in[e])
        y = sum_e combine[n,e]  * (h @ W_out[e] + b_out[e])

V4 strategy (expert parallelism, host-side all-to-all dispatch):
  Core e owns expert e. The host computes the (tiny, 67 MFLOP) router,
  gathers each expert's routed tokens, pre-scales each token row by its
  combine prob p (valid since p>0: p*relu(z) == relu(p*z) and the output
  Linear is linear), pads every expert to a common CAP so the SPMD
  program is shape-identical, and scatter-adds the per-expert outputs.

  The device program per core is a dense relu(x @ W_in) @ W_out with the
  WEIGHTS as the stationary matmul operand and the tokens as the moving
  (free) axis:
    mm1:  hT[ftile, t] += W_in[kd, ftile].T @ xT[kd, t]   (accum over kd)
    mm2:  yT[dtile, t] += W_out[ftc, dtile].T @ hT[ftc, t] (accum over ftc)
  so mm1's output is already transposed for mm2 -> ZERO PE transposes and
  the PE stream is nothing but back-to-back fp16 matmuls. x is gathered /
  transposed / fp16-cast on the host; weights are host-pretiled so every
  DMA row is >=2KB contiguous.

V1 fallback (dense over experts, data parallel) retained for nonzero
b_in/b_out inputs.
"""

import os
import sys

import numpy as np

sys.path.insert(0, "/opt/trn_rl_repo")

import concourse.bacc as bacc
import concourse.bass as bass
import concourse.mybir as mybir
import concourse.tile as tile
from concourse.bass_utils import run_bass_kernel_spmd

F32 = mybir.dt.float32
F32R = mybir.dt.float32r
F16 = mybir.dt.float16

P = 128          # partitions
NCORES = 8
N_TOK = 4096     # total tokens (4*1024)
T = N_TOK // NCORES   # tokens per core = 512 (v1 path)
G = T // P       # token groups per core = 4 (v1 path)
D = 1024
KD = D // P      # 8 contraction chunks for D
F = 4096
FC = F // 512    # 8 f-chunks of 512 (v1 path)
FT = F // P      # 32 f-tiles of 128
DT = D // P      # 8 d-tiles of 128
E = 8
NT = N_TOK
AX = mybir.AxisListType
AF = mybir.ActivationFunctionType
OP = mybir.AluOpType


# ====================================================================
# V4: expert-parallel, host-dispatched, transpose-free.
# ====================================================================


def _chunks(cap):
    """Split cap token columns into <=512-wide PSUM-bank chunks.

    All-but-last chunks are 512 wide; the last carries the remainder so
    the final output copy + DMA on the critical tail is small.
    """
    nch = -(-cap // 512)
    sizes = [512] * (nch - 1) + [cap - 512 * (nch - 1)]
    offs = [0]
    for s in sizes:
        offs.append(offs[-1] + s)
    return nch, sizes, offs


# PE warm-up: dependency-free matmuls bridging the initial weight/x DMA
# so the tensor engine's p-state ramp (cost model: 3us of continuous
# execution) completes before the first real matmul issues.
WARM_N = 64
WARM_COUNT = int(os.environ.get("MOE_WARM", "130"))


def build_nc_v4(cap):
    nch, sizes, offs = _chunks(cap)
    nc = bacc.Bacc(None)
    xT_h = nc.declare_dram_parameter("xT", [D, cap], F16, isOutput=False)
    wi_h = nc.declare_dram_parameter("wi", [FT, P, KD * P], F16, isOutput=False)
    wo_h = nc.declare_dram_parameter("wo", [DT, P, FT * P], F16, isOutput=False)
    yt_h = nc.declare_dram_parameter("yt", [D, cap], F16, isOutput=True)

    with tile.TileContext(nc) as tc:
        with (
            tc.tile_pool(name="persist", bufs=1) as pp,
            tc.tile_pool(name="ps", bufs=8, space="PSUM") as psp,
            tc.tile_pool(name="wi", bufs=3) as wip,
            tc.tile_pool(name="wo", bufs=2) as wop,
            tc.tile_pool(name="yt", bufs=2) as ytp,
        ):
            xT = pp.tile([P, KD, cap], F16, tag="xT")
            hT = pp.tile([P, FT, cap], F16, tag="hT")

            # first weight tile + x chunk-major, in the order mm1 (chunk-
            # outermost) consumes them: chunk 0 of every kd lands first.
            wi_sb0 = wip.tile([P, KD, P], F16, tag="wi", name="wi0")
            nc.sync.dma_start(
                wi_sb0[:], wi_h[0].rearrange("p (kd f) -> p kd f", kd=KD)
            )
            for ch in range(nch):
                o, w = offs[ch], sizes[ch]
                for kd in range(KD):
                    nc.sync.dma_start(
                        xT[:, kd, o : o + w],
                        xT_h[kd * P : (kd + 1) * P, o : o + w],
                    )

            # dependency-free PE warm-up while the DMAs land
            wsrc = pp.tile([P, WARM_N], F16, tag="wsrc")
            nc.vector.memset(wsrc[:], 0.0)
            wps = psp.tile([P, 512], F32, tag="ps", name="wps")
            for _ in range(WARM_COUNT):
                nc.tensor.matmul(
                    wps[:WARM_N, :WARM_N],
                    lhsT=wsrc[:, :],
                    rhs=wsrc[:, :],
                    start=True,
                    stop=True,
                )

            # mm1: hT[ft, t] = relu(sum_kd W_in[kd, ft].T @ xT[kd, t])
            # chunk-outermost: the first full-ft sweep only needs chunk 0
            # of xT, so the PE can start ~4us earlier. W_in is re-streamed
            # per chunk pass (DMA has plenty of headroom).
            for ch in range(nch):
                o, w = offs[ch], sizes[ch]
                for ft in range(FT):
                    if ch == 0 and ft == 0:
                        wi_sb = wi_sb0
                    else:
                        wi_sb = wip.tile([P, KD, P], F16, tag="wi", name="wi")
                        nc.sync.dma_start(
                            wi_sb[:],
                            wi_h[ft].rearrange("p (kd f) -> p kd f", kd=KD),
                        )
                    ps = psp.tile([P, 512], F32, tag="ps", name="ps1")
                    for kd in range(KD):
                        nc.tensor.matmul(
                            ps[:, :w],
                            lhsT=wi_sb[:, kd, :],
                            rhs=xT[:, kd, o : o + w],
                            start=(kd == 0),
                            stop=(kd == KD - 1),
                        )
                    nc.scalar.activation(
                        hT[:, ft, o : o + w], ps[:, :w], AF.Relu
                    )

            # mm2: yT[dt, t] = sum_ftc W_out[ftc, dt].T @ hT[ftc, t]
            for dt in range(DT):
                wo_sb = wop.tile([P, FT, P], F16, tag="wo", name="wo")
                nc.sync.dma_start(
                    wo_sb[:], wo_h[dt].rearrange("p (ftc d) -> p ftc d", ftc=FT)
                )
                yt = ytp.tile([P, cap], F16, tag="yt", name="yt")
                split_out = dt == DT - 1
                for ch in range(nch):
                    o, w = offs[ch], sizes[ch]
                    ps = psp.tile([P, 512], F32, tag="ps", name="ps2")
                    for ftc in range(FT):
                        nc.tensor.matmul(
                            ps[:, :w],
                            lhsT=wo_sb[:, ftc, :],
                            rhs=hT[:, ftc, o : o + w],
                            start=(ftc == 0),
                            stop=(ftc == FT - 1),
                        )
                    nc.vector.tensor_copy(yt[:, o : o + w], ps[:, :w])
                    if split_out:
                        nc.sync.dma_start(
                            yt_h[dt * P : (dt + 1) * P, o : o + w],
                            yt[:, o : o + w],
                        )
                if not split_out:
                    nc.sync.dma_start(yt_h[dt * P : (dt + 1) * P, :], yt[:])

    nc.compile()
    return nc


def route_v4(xf, Wr, br):
    """Host router: per-expert token index lists + combine probs."""
    logits = xf @ np.asarray(Wr, np.float32) + np.asarray(
        br, np.float32
    ).reshape(1, E)
    order = np.argsort(-logits, axis=-1, kind="stable")
    top2 = order[:, :2]
    mx = logits.max(axis=-1, keepdims=True)
    ex = np.exp(logits - mx)
    probs = ex / ex.sum(axis=-1, keepdims=True)
    idx_list, p_list = [], []
    for e in range(E):
        sel = np.nonzero((top2 == e).any(axis=1))[0]
        idx_list.append(sel)
        p_list.append(probs[sel, e].astype(np.float32))
    cap = max(16, max(len(s) for s in idx_list))
    cap = -(-cap // 2) * 2
    return idx_list, p_list, cap


def make_in_maps_v4(x, W_in, W_out, idx_list, p_list, cap):
    xf = np.asarray(x, np.float32).reshape(NT, D)
    in_maps = []
    for e in range(E):
        sel = idx_list[e]
        xs = np.zeros((cap, D), np.float32)
        xs[: len(sel)] = xf[sel] * p_list[e][:, None]
        xT = np.ascontiguousarray(xs.T.astype(np.float16))
        wi = np.ascontiguousarray(
            np.asarray(W_in[e], np.float16)
            .reshape(KD, P, FT, P)
            .transpose(2, 1, 0, 3)
        ).reshape(FT, P, KD * P)
        wo = np.ascontiguousarray(
            np.asarray(W_out[e], np.float16)
            .reshape(FT, P, DT, P)
            .transpose(2, 1, 0, 3)
        ).reshape(DT, P, FT * P)
        in_maps.append({"xT": xT, "wi": wi, "wo": wo})
    return in_maps


# ====================================================================
# V1: dense-over-experts data-parallel fallback (handles any biases).
# ====================================================================


def build_nc(cfg):
    """Build the single-core SPMD bass program (dense over experts).

    cfg keys: wdt ('f32r'|'f16') - dtype of expert weights + hT in matmuls;
              has_br/has_bin/has_bout - include bias adds.
    """
    wdt = F32R if cfg["wdt"] == "f32r" else F16
    w_store = F32R if cfg["wdt"] == "f32r" else F16
    has_br = cfg["has_br"]
    has_bin = cfg["has_bin"]
    has_bout = cfg["has_bout"]

    nc = bacc.Bacc(None)
    x_h = nc.declare_dram_parameter("x", [T, D], F32, isOutput=False)
    wr_h = nc.declare_dram_parameter("wr", [D, E], F32, isOutput=False)
    win_h = nc.declare_dram_parameter("w_in", [E, D, F], w_store, isOutput=False)
    wout_h = nc.declare_dram_parameter("w_out", [E, F, D], w_store, isOutput=False)
    br_h = nc.declare_dram_parameter("br", [1, E], F32, isOutput=False) if has_br else None
    bin_h = nc.declare_dram_parameter("b_in", [E, F], F32, isOutput=False) if has_bin else None
    bout_h = nc.declare_dram_parameter("b_out", [E, D], F32, isOutput=False) if has_bout else None
    y_h = nc.declare_dram_parameter("y", [T, D], F32, isOutput=True)

    with tile.TileContext(nc) as tc:
        with (
            tc.tile_pool(name="persist", bufs=1) as pp,
            tc.tile_pool(name="ps", bufs=6, space="PSUM") as psp,
        ):
            ident = pp.tile([P, P], F32, tag="ident")
            from concourse.masks import make_identity
            make_identity(nc, ident[:])

            xT = pp.tile([P, KD, T], F32, tag="xT")          # x transposed, f32
            hT = pp.tile([P, FT, T], w_store, tag="hT")      # h transposed
            xTr = pp.tile([P, KD, T], w_store, tag="xTr", name="xTr")
            wr_sb = pp.tile([P, KD, E], F32, tag="wr")
            disp = pp.tile([P, G * E], F32, tag="disp")      # dispatch mask
            comb = pp.tile([P, G * E], F32, tag="comb")      # combine probs
            yac = [
                pp.tile([P, D], F32, tag=f"y{g}", name=f"yac{g}")
                for g in range(G)
            ]
            ones1 = pp.tile([1, P], F32, tag="ones1")
            if has_bin or has_bout:
                nc.vector.memset(ones1[:], 1.0)
            br_sb = None
            if has_br:
                br_sb = pp.tile([1, E], F32, tag="br")
                nc.sync.dma_start(br_sb[:], br_h[:])

            nc.sync.dma_start(
                wr_sb[:], wr_h[:, :].rearrange("(kd p) e -> p kd e", p=P)
            )

            with tc.tile_pool(name="xload", bufs=2) as xlp:
                for g in range(G):
                    xg = xlp.tile([P, D], F32, tag="xg")
                    nc.sync.dma_start(xg[:], x_h[g * P : (g + 1) * P, :])
                    for kd in range(KD):
                        pst = psp.tile([P, P], F32, tag="ps")
                        nc.tensor.transpose(
                            pst[:], xg[:, kd * P : (kd + 1) * P], ident[:]
                        )
                        nc.vector.tensor_copy(
                            xT[:, kd, g * P : (g + 1) * P], pst[:]
                        )
                        nc.vector.tensor_copy(
                            xTr[:, kd, g * P : (g + 1) * P], pst[:]
                        )

            # router (true fp32 matmul; top-2 must match reference)
            with tc.tile_pool(name="rt", bufs=2) as rtp:
                for g in range(G):
                    psr = psp.tile([P, E], F32, tag="ps")
                    for kd in range(KD):
                        nc.tensor.matmul(
                            psr[:],
                            lhsT=xT[:, kd, g * P : (g + 1) * P],
                            rhs=wr_sb[:, kd, :],
                            start=(kd == 0),
                            stop=(kd == KD - 1 and not has_br),
                        )
                    if has_br:
                        nc.tensor.matmul(
                            psr[:], lhsT=ones1[:, :], rhs=br_sb[:, :],
                            start=False, stop=True,
                        )
                    lg = rtp.tile([P, E], F32, tag="lg")
                    nc.vector.tensor_copy(lg[:], psr[:])
                    mx1 = rtp.tile([P, 1], F32, tag="mx1")
                    nmx = rtp.tile([P, 1], F32, tag="nmx")
                    nc.vector.reduce_max(out=mx1[:], in_=lg[:], axis=AX.X)
                    nc.vector.reduce_max(out=nmx[:], in_=lg[:], axis=AX.X, negate=True)
                    is1 = rtp.tile([P, E], F32, tag="is1")
                    nc.vector.tensor_scalar(
                        out=is1[:], in0=lg[:], scalar1=mx1[:, :1], scalar2=None,
                        op0=OP.is_equal,
                    )
                    lgm = rtp.tile([P, E], F32, tag="lgm")
                    nc.vector.tensor_scalar_mul(is1[:], is1[:], 1e30)
                    nc.vector.tensor_sub(lgm[:], lg[:], is1[:])
                    mx2 = rtp.tile([P, 1], F32, tag="mx2")
                    nc.vector.reduce_max(out=mx2[:], in_=lgm[:], axis=AX.X)
                    dcol = disp[:, g * E : (g + 1) * E]
                    nc.vector.tensor_scalar(
                        out=dcol, in0=lg[:], scalar1=mx2[:, :1], scalar2=None,
                        op0=OP.is_ge,
                    )
                    ex = rtp.tile([P, E], F32, tag="ex")
                    nc.scalar.activation(ex[:], lg[:], AF.Exp, bias=nmx[:, :1])
                    sm = rtp.tile([P, 1], F32, tag="sm")
                    nc.vector.reduce_sum(out=sm[:], in_=ex[:], axis=AX.X)
                    rc = rtp.tile([P, 1], F32, tag="rc")
                    nc.vector.reciprocal(rc[:], sm[:])
                    nc.vector.tensor_scalar_mul(ex[:], ex[:], rc[:, :1])
                    nc.vector.tensor_mul(
                        comb[:, g * E : (g + 1) * E], ex[:], dcol
                    )

            # mm1: h = sum_e mask_e * relu(x@W_in[e] (+ b_in))
            with (
                tc.tile_pool(name="wfe", bufs=2) as wfp,
                tc.tile_pool(name="hf", bufs=2 * G) as hfp,
                tc.tile_pool(name="rtmp", bufs=4) as rtmp,
            ):
                for f in range(FC):
                    hfs = []
                    for e in range(E):
                        wfe = wfp.tile([P, KD, 512], w_store, tag="wfe")
                        nc.sync.dma_start(
                            wfe[:],
                            win_h[e, :, f * 512 : (f + 1) * 512].rearrange(
                                "(kd p) f -> p kd f", p=P
                            ),
                        )
                        if has_bin:
                            bin_sb = wfp.tile([1, 512], F32, tag="bin")
                            nc.sync.dma_start(
                                bin_sb[:],
                                bin_h[e, f * 512 : (f + 1) * 512][None, :],
                            )
                        for g in range(G):
                            ps = psp.tile([P, 512], F32, tag="ps")
                            for kd in range(KD):
                                nc.tensor.matmul(
                                    ps[:],
                                    lhsT=xTr[:, kd, g * P : (g + 1) * P],
                                    rhs=wfe[:, kd, :],
                                    start=(kd == 0),
                                    stop=(kd == KD - 1 and not has_bin),
                                )
                            if has_bin:
                                nc.tensor.matmul(
                                    ps[:],
                                    lhsT=ones1[:, :],
                                    rhs=bin_sb[:, :],
                                    start=False, stop=True,
                                )
                            sc = disp[:, g * E + e : g * E + e + 1]
                            if e == 0:
                                hf = hfp.tile([P, 512], F32, tag="hf")
                                hfs.append(hf)
                                nc.scalar.activation(
                                    hf[:], ps[:], AF.Relu, scale=sc
                                )
                            else:
                                tmp = rtmp.tile([P, 512], F32, tag="rtmp")
                                nc.scalar.activation(
                                    tmp[:], ps[:], AF.Relu, scale=sc
                                )
                                nc.vector.tensor_add(hfs[g][:], hfs[g][:], tmp[:])
                    for g in range(G):
                        for c in range(4):
                            pst = psp.tile([P, P], F32, tag="ps")
                            nc.tensor.transpose(
                                pst[:],
                                hfs[g][:, c * P : (c + 1) * P],
                                ident[:],
                            )
                            nc.vector.tensor_copy(
                                hT[:, f * 4 + c, g * P : (g + 1) * P], pst[:]
                            )

            # mm2: y = sum_e comb_e * (h@W_out[e] (+ b_out))
            ndh = 2 if wdt == F16 else 4
            dw = D // ndh
            with tc.tile_pool(name="wo", bufs=2) as wop:
                for e in range(E):
                    for dh in range(ndh):
                        wo = wop.tile([P, FT, dw], w_store, tag="wo")
                        nc.sync.dma_start(
                            wo[:],
                            wout_h[e, :, dh * dw : (dh + 1) * dw].rearrange(
                                "(ft p) d -> p ft d", p=P
                            ),
                        )
                        if has_bout:
                            bout_sb = wop.tile([1, dw], F32, tag="bout")
                            nc.sync.dma_start(
                                bout_sb[:],
                                bout_h[e, dh * dw : (dh + 1) * dw][None, :],
                            )
                        for g in range(G):
                            ps = psp.tile([P, dw], F32, tag="ps")
                            for ft in range(FT):
                                nc.tensor.matmul(
                                    ps[:],
                                    lhsT=hT[:, ft, g * P : (g + 1) * P],
                                    rhs=wo[:, ft, :],
                                    start=(ft == 0),
                                    stop=(ft == FT - 1 and not has_bout),
                                )
                            if has_bout:
                                nc.tensor.matmul(
                                    ps[:],
                                    lhsT=ones1[:, :],
                                    rhs=bout_sb[:, :],
                                    start=False, stop=True,
                                )
                            cc = comb[:, g * E + e : g * E + e + 1]
                            ysl = yac[g][:, dh * dw : (dh + 1) * dw]
                            if e == 0:
                                nc.vector.tensor_scalar(
                                    out=ysl, in0=ps[:], scalar1=cc,
                                    scalar2=None, op0=OP.mult,
                                )
                            else:
                                tm = wop.tile([P, dw], F32, tag="ytmp")
                                nc.vector.tensor_scalar(
                                    out=tm[:], in0=ps[:], scalar1=cc,
                                    scalar2=None, op0=OP.mult,
                                )
                                nc.vector.tensor_add(ysl, ysl, tm[:])

            for g in range(G):
                nc.sync.dma_start(y_h[g * P : (g + 1) * P, :], yac[g][:])

    nc.compile()
    return nc


_NC_CACHE = {}


def get_nc(cfg_key):
    if cfg_key not in _NC_CACHE:
        cfg = dict(
            wdt=cfg_key[0], has_br=cfg_key[1], has_bin=cfg_key[2],
            has_bout=cfg_key[3],
        )
        _NC_CACHE[cfg_key] = build_nc(cfg)
    return _NC_CACHE[cfg_key]


def get_nc_v4(cap):
    key = ("v4", cap)
    if key not in _NC_CACHE:
        _NC_CACHE[key] = build_nc_v4(cap)
    return _NC_CACHE[key]


WDT_MODE = os.environ.get("MOE_WDT", "f16")


def make_in_maps(x, Wr, br, W_in, b_in, W_out, b_out, wdt_mode):
    xf = np.ascontiguousarray(np.asarray(x, np.float32).reshape(N_TOK, D))
    w_store_np = np.float32 if wdt_mode == "f32r" else np.float16
    win = np.ascontiguousarray(np.asarray(W_in, w_store_np))
    wout = np.ascontiguousarray(np.asarray(W_out, w_store_np))
    wr = np.ascontiguousarray(np.asarray(Wr, np.float32))
    has_br = bool(np.any(np.asarray(br) != 0))
    has_bin = bool(np.any(np.asarray(b_in) != 0))
    has_bout = bool(np.any(np.asarray(b_out) != 0))
    in_maps = []
    for c in range(NCORES):
        m = {
            "x": xf[c * T : (c + 1) * T],
            "wr": wr,
            "w_in": win,
            "w_out": wout,
        }
        if has_br:
            m["br"] = np.asarray(br, np.float32).reshape(1, E)
        if has_bin:
            m["b_in"] = np.asarray(b_in, np.float32)
        if has_bout:
            m["b_out"] = np.asarray(b_out, np.float32)
        in_maps.append(m)
    cfg_key = (wdt_mode, has_br, has_bin, has_bout)
    return cfg_key, in_maps


# v4 = expert-parallel host-dispatched (default); v1 = dense fallback
# (v1 also serves as the general path when b_in/b_out is nonzero)
IMPL = os.environ.get("MOE_IMPL", "v4")


def kernel(x, Wr, br, W_in, b_in, W_out, b_out, top_k):
    assert int(top_k) == 2, "kernel is specialized for top_k=2"
    if IMPL == "v4" and not (np.any(np.asarray(b_in)) or np.any(np.asarray(b_out))):
        xf = np.ascontiguousarray(np.asarray(x, np.float32).reshape(NT, D))
        idx_list, p_list, cap = route_v4(xf, Wr, br)
        in_maps = make_in_maps_v4(x, W_in, W_out, idx_list, p_list, cap)
        nc = get_nc_v4(cap)
        res = run_bass_kernel_spmd(nc, in_maps, list(range(NCORES)))
        y = np.zeros((NT, D), np.float32)
        for e in range(E):
            n = len(idx_list[e])
            ye = np.asarray(res.results[e]["yt"])  # [D, cap] f16
            y[idx_list[e]] += ye[:, :n].T.astype(np.float32)
        return y.reshape(4, 1024, 1024)
    cfg_key, in_maps = make_in_maps(
        x, Wr, br, W_in, b_in, W_out, b_out, WDT_MODE
    )
    nc = get_nc(cfg_key)
    res = run_bass_kernel_spmd(nc, in_maps, list(range(NCORES)))
    y = np.concatenate([res.results[c]["y"] for c in range(NCORES)], axis=0)
    return y.reshape(4, 1024, 1024).astype(np.float32)


# revision 9
# speedup vs baseline: 1.1055x; 1.1055x over previous
"""MoE feed-forward (top-2 of 8 experts) Trainium2 Bass kernel.

Problem: nn_MixtureOfExpertsFeedForward_6734508720763
  x[4,1024,1024] tokens, router Wr[1024,8], experts W_in[8,1024,4096],
  W_out[8,4096,1024], top_k=2.

  ref:  logits = x@Wr + br ; probs = softmax(logits)
        top2 -> dispatch (0/1), combine (prob or 0)
        h = sum_e dispatch[n,e] * relu(x @ W_in[e] + b_in[e])
        y = sum_e combine[n,e]  * (h @ W_out[e] + b_out[e])

V4 strategy (expert parallelism, host-side all-to-all dispatch):
  Core e owns expert e. The host computes the (tiny, 67 MFLOP) router,
  gathers each expert's routed tokens, pre-scales each token row by its
  combine prob p (valid since p>0: p*relu(z) == relu(p*z) and the output
  Linear is linear), pads every expert to a common CAP so the SPMD
  program is shape-identical, and scatter-adds the per-expert outputs.

  The device program per core is a dense relu(x @ W_in) @ W_out with the
  WEIGHTS as the stationary matmul operand and the tokens as the moving
  (free) axis:
    mm1:  hT[ftile, t] += W_in[kd, ftile].T @ xT[kd, t]   (accum over kd)
    mm2:  yT[dtile, t] += W_out[ftc, dtile].T @ hT[ftc, t] (accum over ftc)
  so mm1's output is already transposed for mm2 -> ZERO PE transposes and
  the PE stream is nothing but back-to-back fp16 matmuls. x is gathered /
  transposed / fp16-cast on the host; weights are host-pretiled so every
  DMA row is >=2KB contiguous.

V1 fallback (dense over experts, data parallel) retained for nonzero
b_in/b_out inputs.
"""

import os
import sys

import numpy as np

sys.path.insert(0, "/opt/trn_rl_repo")

import concourse.bacc as bacc
import concourse.bass as bass
import concourse.mybir as mybir
import concourse.tile as tile
from concourse.bass_utils import run_bass_kernel_spmd

F32 = mybir.dt.float32
F32R = mybir.dt.float32r
F16 = mybir.dt.float16

P = 128          # partitions
NCORES = 8
N_TOK = 4096     # total tokens (4*1024)
T = N_TOK // NCORES   # tokens per core = 512 (v1 path)
G = T // P       # token groups per core = 4 (v1 path)
D = 1024
KD = D // P      # 8 contraction chunks for D
F = 4096
FC = F // 512    # 8 f-chunks of 512 (v1 path)
FT = F // P      # 32 f-tiles of 128
DT = D // P      # 8 d-tiles of 128
E = 8
NT = N_TOK
AX = mybir.AxisListType
AF = mybir.ActivationFunctionType
OP = mybir.AluOpType


# ====================================================================
# V4: expert-parallel, host-dispatched, transpose-free.
# ====================================================================


def _chunks(cap):
    """Split cap token columns into <=512-wide PSUM-bank chunks.

    All-but-last chunks are 512 wide; the last carries the remainder so
    the final output copy + DMA on the critical tail is small.
    """
    nch = -(-cap // 512)
    sizes = [512] * (nch - 1) + [cap - 512 * (nch - 1)]
    offs = [0]
    for s in sizes:
        offs.append(offs[-1] + s)
    return nch, sizes, offs


# PE warm-up: dependency-free matmuls bridging the initial weight/x DMA
# so the tensor engine's p-state ramp (cost model: 3us of continuous
# execution) completes before the first real matmul issues.
WARM_N = 64
WARM_COUNT = int(os.environ.get("MOE_WARM", "130"))


def build_nc_v4(cap):
    nch, sizes, offs = _chunks(cap)
    nc = bacc.Bacc(None)
    xT_h = nc.declare_dram_parameter("xT", [D, cap], F16, isOutput=False)
    wi_h = nc.declare_dram_parameter("wi", [FT, P, KD * P], F16, isOutput=False)
    wo_h = nc.declare_dram_parameter("wo", [DT, P, FT * P], F16, isOutput=False)
    yt_h = nc.declare_dram_parameter("yt", [D, cap], F16, isOutput=True)

    with tile.TileContext(nc) as tc:
        with (
            tc.tile_pool(name="persist", bufs=1) as pp,
            tc.tile_pool(name="ps", bufs=8, space="PSUM") as psp,
            tc.tile_pool(name="wi", bufs=3) as wip,
            tc.tile_pool(name="wo", bufs=2) as wop,
            tc.tile_pool(name="yt", bufs=2) as ytp,
        ):
            xT = pp.tile([P, KD, cap], F16, tag="xT")
            hT = pp.tile([P, FT, cap], F16, tag="hT")

            # first weight tile + x chunk-major, in the order mm1 (chunk-
            # outermost) consumes them: chunk 0 of every kd lands first.
            wi_sb0 = wip.tile([P, KD, P], F16, tag="wi", name="wi0")
            nc.sync.dma_start(
                wi_sb0[:], wi_h[0].rearrange("p (kd f) -> p kd f", kd=KD)
            )
            for ch in range(nch):
                o, w = offs[ch], sizes[ch]
                for kd in range(KD):
                    nc.sync.dma_start(
                        xT[:, kd, o : o + w],
                        xT_h[kd * P : (kd + 1) * P, o : o + w],
                    )

            # dependency-free PE warm-up while the DMAs land
            wsrc = pp.tile([P, WARM_N], F16, tag="wsrc")
            nc.vector.memset(wsrc[:], 0.0)
            wps = psp.tile([P, 512], F32, tag="ps", name="wps")
            for _ in range(WARM_COUNT):
                nc.tensor.matmul(
                    wps[:WARM_N, :WARM_N],
                    lhsT=wsrc[:, :],
                    rhs=wsrc[:, :],
                    start=True,
                    stop=True,
                )

            # mm1: hT[ft, t] = relu(sum_kd W_in[kd, ft].T @ xT[kd, t])
            # Two ft-sweep passes: chunk 0 alone first (so the PE can start
            # after only chunk 0 of xT lands, ~4us earlier), then the
            # remaining chunks together (keeps every pass PE-bound: the
            # remainder chunk alone would be DMA-bound on the wi stream).
            passes = [[0], list(range(1, nch))] if nch > 1 else [[0]]
            for chs in passes:
                for ft in range(FT):
                    if chs[0] == 0 and ft == 0:
                        wi_sb = wi_sb0
                    else:
                        wi_sb = wip.tile([P, KD, P], F16, tag="wi", name="wi")
                        nc.sync.dma_start(
                            wi_sb[:],
                            wi_h[ft].rearrange("p (kd f) -> p kd f", kd=KD),
                        )
                    pss = [
                        psp.tile([P, 512], F32, tag="ps", name=f"ps1_{ch}")
                        for ch in chs
                    ]
                    for kd in range(KD):
                        for ps, ch in zip(pss, chs):
                            o, w = offs[ch], sizes[ch]
                            nc.tensor.matmul(
                                ps[:, :w],
                                lhsT=wi_sb[:, kd, :],
                                rhs=xT[:, kd, o : o + w],
                                start=(kd == 0),
                                stop=(kd == KD - 1),
                            )
                    for ps, ch in zip(pss, chs):
                        o, w = offs[ch], sizes[ch]
                        nc.scalar.activation(
                            hT[:, ft, o : o + w], ps[:, :w], AF.Relu
                        )

            # mm2: yT[dt, t] = sum_ftc W_out[ftc, dt].T @ hT[ftc, t]
            for dt in range(DT):
                wo_sb = wop.tile([P, FT, P], F16, tag="wo", name="wo")
                nc.sync.dma_start(
                    wo_sb[:], wo_h[dt].rearrange("p (ftc d) -> p ftc d", ftc=FT)
                )
                yt = ytp.tile([P, cap], F16, tag="yt", name="yt")
                split_out = dt == DT - 1
                for ch in range(nch):
                    o, w = offs[ch], sizes[ch]
                    ps = psp.tile([P, 512], F32, tag="ps", name="ps2")
                    for ftc in range(FT):
                        nc.tensor.matmul(
                            ps[:, :w],
                            lhsT=wo_sb[:, ftc, :],
                            rhs=hT[:, ftc, o : o + w],
                            start=(ftc == 0),
                            stop=(ftc == FT - 1),
                        )
                    nc.vector.tensor_copy(yt[:, o : o + w], ps[:, :w])
                    if split_out:
                        nc.sync.dma_start(
                            yt_h[dt * P : (dt + 1) * P, o : o + w],
                            yt[:, o : o + w],
                        )
                if not split_out:
                    nc.sync.dma_start(yt_h[dt * P : (dt + 1) * P, :], yt[:])

    nc.compile()
    return nc


def route_v4(xf, Wr, br):
    """Host router: per-expert token index lists + combine probs."""
    logits = xf @ np.asarray(Wr, np.float32) + np.asarray(
        br, np.float32
    ).reshape(1, E)
    order = np.argsort(-logits, axis=-1, kind="stable")
    top2 = order[:, :2]
    mx = logits.max(axis=-1, keepdims=True)
    ex = np.exp(logits - mx)
    probs = ex / ex.sum(axis=-1, keepdims=True)
    idx_list, p_list = [], []
    for e in range(E):
        sel = np.nonzero((top2 == e).any(axis=1))[0]
        idx_list.append(sel)
        p_list.append(probs[sel, e].astype(np.float32))
    cap = max(16, max(len(s) for s in idx_list))
    cap = -(-cap // 2) * 2
    return idx_list, p_list, cap


def make_in_maps_v4(x, W_in, W_out, idx_list, p_list, cap):
    xf = np.asarray(x, np.float32).reshape(NT, D)
    in_maps = []
    for e in range(E):
        sel = idx_list[e]
        xs = np.zeros((cap, D), np.float32)
        xs[: len(sel)] = xf[sel] * p_list[e][:, None]
        xT = np.ascontiguousarray(xs.T.astype(np.float16))
        wi = np.ascontiguousarray(
            np.asarray(W_in[e], np.float16)
            .reshape(KD, P, FT, P)
            .transpose(2, 1, 0, 3)
        ).reshape(FT, P, KD * P)
        wo = np.ascontiguousarray(
            np.asarray(W_out[e], np.float16)
            .reshape(FT, P, DT, P)
            .transpose(2, 1, 0, 3)
        ).reshape(DT, P, FT * P)
        in_maps.append({"xT": xT, "wi": wi, "wo": wo})
    return in_maps


# ====================================================================
# V1: dense-over-experts data-parallel fallback (handles any biases).
# ====================================================================


def build_nc(cfg):
    """Build the single-core SPMD bass program (dense over experts).

    cfg keys: wdt ('f32r'|'f16') - dtype of expert weights + hT in matmuls;
              has_br/has_bin/has_bout - include bias adds.
    """
    wdt = F32R if cfg["wdt"] == "f32r" else F16
    w_store = F32R if cfg["wdt"] == "f32r" else F16
    has_br = cfg["has_br"]
    has_bin = cfg["has_bin"]
    has_bout = cfg["has_bout"]

    nc = bacc.Bacc(None)
    x_h = nc.declare_dram_parameter("x", [T, D], F32, isOutput=False)
    wr_h = nc.declare_dram_parameter("wr", [D, E], F32, isOutput=False)
    win_h = nc.declare_dram_parameter("w_in", [E, D, F], w_store, isOutput=False)
    wout_h = nc.declare_dram_parameter("w_out", [E, F, D], w_store, isOutput=False)
    br_h = nc.declare_dram_parameter("br", [1, E], F32, isOutput=False) if has_br else None
    bin_h = nc.declare_dram_parameter("b_in", [E, F], F32, isOutput=False) if has_bin else None
    bout_h = nc.declare_dram_parameter("b_out", [E, D], F32, isOutput=False) if has_bout else None
    y_h = nc.declare_dram_parameter("y", [T, D], F32, isOutput=True)

    with tile.TileContext(nc) as tc:
        with (
            tc.tile_pool(name="persist", bufs=1) as pp,
            tc.tile_pool(name="ps", bufs=6, space="PSUM") as psp,
        ):
            ident = pp.tile([P, P], F32, tag="ident")
            from concourse.masks import make_identity
            make_identity(nc, ident[:])

            xT = pp.tile([P, KD, T], F32, tag="xT")          # x transposed, f32
            hT = pp.tile([P, FT, T], w_store, tag="hT")      # h transposed
            xTr = pp.tile([P, KD, T], w_store, tag="xTr", name="xTr")
            wr_sb = pp.tile([P, KD, E], F32, tag="wr")
            disp = pp.tile([P, G * E], F32, tag="disp")      # dispatch mask
            comb = pp.tile([P, G * E], F32, tag="comb")      # combine probs
            yac = [
                pp.tile([P, D], F32, tag=f"y{g}", name=f"yac{g}")
                for g in range(G)
            ]
            ones1 = pp.tile([1, P], F32, tag="ones1")
            if has_bin or has_bout:
                nc.vector.memset(ones1[:], 1.0)
            br_sb = None
            if has_br:
                br_sb = pp.tile([1, E], F32, tag="br")
                nc.sync.dma_start(br_sb[:], br_h[:])

            nc.sync.dma_start(
                wr_sb[:], wr_h[:, :].rearrange("(kd p) e -> p kd e", p=P)
            )

            with tc.tile_pool(name="xload", bufs=2) as xlp:
                for g in range(G):
                    xg = xlp.tile([P, D], F32, tag="xg")
                    nc.sync.dma_start(xg[:], x_h[g * P : (g + 1) * P, :])
                    for kd in range(KD):
                        pst = psp.tile([P, P], F32, tag="ps")
                        nc.tensor.transpose(
                            pst[:], xg[:, kd * P : (kd + 1) * P], ident[:]
                        )
                        nc.vector.tensor_copy(
                            xT[:, kd, g * P : (g + 1) * P], pst[:]
                        )
                        nc.vector.tensor_copy(
                            xTr[:, kd, g * P : (g + 1) * P], pst[:]
                        )

            # router (true fp32 matmul; top-2 must match reference)
            with tc.tile_pool(name="rt", bufs=2) as rtp:
                for g in range(G):
                    psr = psp.tile([P, E], F32, tag="ps")
                    for kd in range(KD):
                        nc.tensor.matmul(
                            psr[:],
                            lhsT=xT[:, kd, g * P : (g + 1) * P],
                            rhs=wr_sb[:, kd, :],
                            start=(kd == 0),
                            stop=(kd == KD - 1 and not has_br),
                        )
                    if has_br:
                        nc.tensor.matmul(
                            psr[:], lhsT=ones1[:, :], rhs=br_sb[:, :],
                            start=False, stop=True,
                        )
                    lg = rtp.tile([P, E], F32, tag="lg")
                    nc.vector.tensor_copy(lg[:], psr[:])
                    mx1 = rtp.tile([P, 1], F32, tag="mx1")
                    nmx = rtp.tile([P, 1], F32, tag="nmx")
                    nc.vector.reduce_max(out=mx1[:], in_=lg[:], axis=AX.X)
                    nc.vector.reduce_max(out=nmx[:], in_=lg[:], axis=AX.X, negate=True)
                    is1 = rtp.tile([P, E], F32, tag="is1")
                    nc.vector.tensor_scalar(
                        out=is1[:], in0=lg[:], scalar1=mx1[:, :1], scalar2=None,
                        op0=OP.is_equal,
                    )
                    lgm = rtp.tile([P, E], F32, tag="lgm")
                    nc.vector.tensor_scalar_mul(is1[:], is1[:], 1e30)
                    nc.vector.tensor_sub(lgm[:], lg[:], is1[:])
                    mx2 = rtp.tile([P, 1], F32, tag="mx2")
                    nc.vector.reduce_max(out=mx2[:], in_=lgm[:], axis=AX.X)
                    dcol = disp[:, g * E : (g + 1) * E]
                    nc.vector.tensor_scalar(
                        out=dcol, in0=lg[:], scalar1=mx2[:, :1], scalar2=None,
                        op0=OP.is_ge,
                    )
                    ex = rtp.tile([P, E], F32, tag="ex")
                    nc.scalar.activation(ex[:], lg[:], AF.Exp, bias=nmx[:, :1])
                    sm = rtp.tile([P, 1], F32, tag="sm")
                    nc.vector.reduce_sum(out=sm[:], in_=ex[:], axis=AX.X)
                    rc = rtp.tile([P, 1], F32, tag="rc")
                    nc.vector.reciprocal(rc[:], sm[:])
                    nc.vector.tensor_scalar_mul(ex[:], ex[:], rc[:, :1])
                    nc.vector.tensor_mul(
                        comb[:, g * E : (g + 1) * E], ex[:], dcol
                    )

            # mm1: h = sum_e mask_e * relu(x@W_in[e] (+ b_in))
            with (
                tc.tile_pool(name="wfe", bufs=2) as wfp,
                tc.tile_pool(name="hf", bufs=2 * G) as hfp,
                tc.tile_pool(name="rtmp", bufs=4) as rtmp,
            ):
                for f in range(FC):
                    hfs = []
                    for e in range(E):
                        wfe = wfp.tile([P, KD, 512], w_store, tag="wfe")
                        nc.sync.dma_start(
                            wfe[:],
                            win_h[e, :, f * 512 : (f + 1) * 512].rearrange(
                                "(kd p) f -> p kd f", p=P
                            ),
                        )
                        if has_bin:
                            bin_sb = wfp.tile([1, 512], F32, tag="bin")
                            nc.sync.dma_start(
                                bin_sb[:],
                                bin_h[e, f * 512 : (f + 1) * 512][None, :],
                            )
                        for g in range(G):
                            ps = psp.tile([P, 512], F32, tag="ps")
                            for kd in range(KD):
                                nc.tensor.matmul(
                                    ps[:],
                                    lhsT=xTr[:, kd, g * P : (g + 1) * P],
                                    rhs=wfe[:, kd, :],
                                    start=(kd == 0),
                                    stop=(kd == KD - 1 and not has_bin),
                                )
                            if has_bin:
                                nc.tensor.matmul(
                                    ps[:],
                                    lhsT=ones1[:, :],
                                    rhs=bin_sb[:, :],
                                    start=False, stop=True,
                                )
                            sc = disp[:, g * E + e : g * E + e + 1]
                            if e == 0:
                                hf = hfp.tile([P, 512], F32, tag="hf")
                                hfs.append(hf)
                                nc.scalar.activation(
                                    hf[:], ps[:], AF.Relu, scale=sc
                                )
                            else:
                                tmp = rtmp.tile([P, 512], F32, tag="rtmp")
                                nc.scalar.activation(
                                    tmp[:], ps[:], AF.Relu, scale=sc
                                )
                                nc.vector.tensor_add(hfs[g][:], hfs[g][:], tmp[:])
                    for g in range(G):
                        for c in range(4):
                            pst = psp.tile([P, P], F32, tag="ps")
                            nc.tensor.transpose(
                                pst[:],
                                hfs[g][:, c * P : (c + 1) * P],
                                ident[:],
                            )
                            nc.vector.tensor_copy(
                                hT[:, f * 4 + c, g * P : (g + 1) * P], pst[:]
                            )

            # mm2: y = sum_e comb_e * (h@W_out[e] (+ b_out))
            ndh = 2 if wdt == F16 else 4
            dw = D // ndh
            with tc.tile_pool(name="wo", bufs=2) as wop:
                for e in range(E):
                    for dh in range(ndh):
                        wo = wop.tile([P, FT, dw], w_store, tag="wo")
                        nc.sync.dma_start(
                            wo[:],
                            wout_h[e, :, dh * dw : (dh + 1) * dw].rearrange(
                                "(ft p) d -> p ft d", p=P
                            ),
                        )
                        if has_bout:
                            bout_sb = wop.tile([1, dw], F32, tag="bout")
                            nc.sync.dma_start(
                                bout_sb[:],
                                bout_h[e, dh * dw : (dh + 1) * dw][None, :],
                            )
                        for g in range(G):
                            ps = psp.tile([P, dw], F32, tag="ps")
                            for ft in range(FT):
                                nc.tensor.matmul(
                                    ps[:],
                                    lhsT=hT[:, ft, g * P : (g + 1) * P],
                                    rhs=wo[:, ft, :],
                                    start=(ft == 0),
                                    stop=(ft == FT - 1 and not has_bout),
                                )
                            if has_bout:
                                nc.tensor.matmul(
                                    ps[:],
                                    lhsT=ones1[:, :],
                                    rhs=bout_sb[:, :],
                                    start=False, stop=True,
                                )
                            cc = comb[:, g * E + e : g * E + e + 1]
                            ysl = yac[g][:, dh * dw : (dh + 1) * dw]
                            if e == 0:
                                nc.vector.tensor_scalar(
                                    out=ysl, in0=ps[:], scalar1=cc,
                                    scalar2=None, op0=OP.mult,
                                )
                            else:
                                tm = wop.tile([P, dw], F32, tag="ytmp")
                                nc.vector.tensor_scalar(
                                    out=tm[:], in0=ps[:], scalar1=cc,
                                    scalar2=None, op0=OP.mult,
                                )
                                nc.vector.tensor_add(ysl, ysl, tm[:])

            for g in range(G):
                nc.sync.dma_start(y_h[g * P : (g + 1) * P, :], yac[g][:])

    nc.compile()
    return nc


_NC_CACHE = {}


def get_nc(cfg_key):
    if cfg_key not in _NC_CACHE:
        cfg = dict(
            wdt=cfg_key[0], has_br=cfg_key[1], has_bin=cfg_key[2],
            has_bout=cfg_key[3],
        )
        _NC_CACHE[cfg_key] = build_nc(cfg)
    return _NC_CACHE[cfg_key]


def get_nc_v4(cap):
    key = ("v4", cap)
    if key not in _NC_CACHE:
        _NC_CACHE[key] = build_nc_v4(cap)
    return _NC_CACHE[key]


WDT_MODE = os.environ.get("MOE_WDT", "f16")


def make_in_maps(x, Wr, br, W_in, b_in, W_out, b_out, wdt_mode):
    xf = np.ascontiguousarray(np.asarray(x, np.float32).reshape(N_TOK, D))
    w_store_np = np.float32 if wdt_mode == "f32r" else np.float16
    win = np.ascontiguousarray(np.asarray(W_in, w_store_np))
    wout = np.ascontiguousarray(np.asarray(W_out, w_store_np))
    wr = np.ascontiguousarray(np.asarray(Wr, np.float32))
    has_br = bool(np.any(np.asarray(br) != 0))
    has_bin = bool(np.any(np.asarray(b_in) != 0))
    has_bout = bool(np.any(np.asarray(b_out) != 0))
    in_maps = []
    for c in range(NCORES):
        m = {
            "x": xf[c * T : (c + 1) * T],
            "wr": wr,
            "w_in": win,
            "w_out": wout,
        }
        if has_br:
            m["br"] = np.asarray(br, np.float32).reshape(1, E)
        if has_bin:
            m["b_in"] = np.asarray(b_in, np.float32)
        if has_bout:
            m["b_out"] = np.asarray(b_out, np.float32)
        in_maps.append(m)
    cfg_key = (wdt_mode, has_br, has_bin, has_bout)
    return cfg_key, in_maps


# v4 = expert-parallel host-dispatched (default); v1 = dense fallback
# (v1 also serves as the general path when b_in/b_out is nonzero)
IMPL = os.environ.get("MOE_IMPL", "v4")


def kernel(x, Wr, br, W_in, b_in, W_out, b_out, top_k):
    assert int(top_k) == 2, "kernel is specialized for top_k=2"
    if IMPL == "v4" and not (np.any(np.asarray(b_in)) or np.any(np.asarray(b_out))):
        xf = np.ascontiguousarray(np.asarray(x, np.float32).reshape(NT, D))
        idx_list, p_list, cap = route_v4(xf, Wr, br)
        in_maps = make_in_maps_v4(x, W_in, W_out, idx_list, p_list, cap)
        nc = get_nc_v4(cap)
        res = run_bass_kernel_spmd(nc, in_maps, list(range(NCORES)))
        y = np.zeros((NT, D), np.float32)
        for e in range(E):
            n = len(idx_list[e])
            ye = np.asarray(res.results[e]["yt"])  # [D, cap] f16
            y[idx_list[e]] += ye[:, :n].T.astype(np.float32)
        return y.reshape(4, 1024, 1024)
    cfg_key, in_maps = make_in_maps(
        x, Wr, br, W_in, b_in, W_out, b_out, WDT_MODE
    )
    nc = get_nc(cfg_key)
    res = run_bass_kernel_spmd(nc, in_maps, list(range(NCORES)))
    y = np.concatenate([res.results[c]["y"] for c in range(NCORES)], axis=0)
    return y.reshape(4, 1024, 1024).astype(np.float32)


# revision 10
# speedup vs baseline: 1.1926x; 1.0787x over previous
"""MoE feed-forward (top-2 of 8 experts) Trainium2 Bass kernel.

Problem: nn_MixtureOfExpertsFeedForward_6734508720763
  x[4,1024,1024] tokens, router Wr[1024,8], experts W_in[8,1024,4096],
  W_out[8,4096,1024], top_k=2.

  ref:  logits = x@Wr + br ; probs = softmax(logits)
        top2 -> dispatch (0/1), combine (prob or 0)
        h = sum_e dispatch[n,e] * relu(x @ W_in[e] + b_in[e])
        y = sum_e combine[n,e]  * (h @ W_out[e] + b_out[e])

V4 strategy (expert parallelism, host-side all-to-all dispatch):
  Core e owns expert e. The host computes the (tiny, 67 MFLOP) router,
  gathers each expert's routed tokens, pre-scales each token row by its
  combine prob p (valid since p>0: p*relu(z) == relu(p*z) and the output
  Linear is linear), pads every expert to a common CAP so the SPMD
  program is shape-identical, and scatter-adds the per-expert outputs.

  The device program per core is a dense relu(x @ W_in) @ W_out with the
  WEIGHTS as the stationary matmul operand and the tokens as the moving
  (free) axis:
    mm1:  hT[ftile, t] += W_in[kd, ftile].T @ xT[kd, t]   (accum over kd)
    mm2:  yT[dtile, t] += W_out[ftc, dtile].T @ hT[ftc, t] (accum over ftc)
  so mm1's output is already transposed for mm2 -> ZERO PE transposes and
  the PE stream is nothing but back-to-back fp16 matmuls. x is gathered /
  transposed / fp16-cast on the host; weights are host-pretiled so every
  DMA row is >=2KB contiguous.

V1 fallback (dense over experts, data parallel) retained for nonzero
b_in/b_out inputs.
"""

import os
import sys

import numpy as np

sys.path.insert(0, "/opt/trn_rl_repo")

import concourse.bacc as bacc
import concourse.bass as bass
import concourse.mybir as mybir
import concourse.tile as tile
from concourse.bass_utils import run_bass_kernel_spmd

F32 = mybir.dt.float32
F32R = mybir.dt.float32r
F16 = mybir.dt.float16

P = 128          # partitions
NCORES = 8
N_TOK = 4096     # total tokens (4*1024)
T = N_TOK // NCORES   # tokens per core = 512 (v1 path)
G = T // P       # token groups per core = 4 (v1 path)
D = 1024
KD = D // P      # 8 contraction chunks for D
F = 4096
FC = F // 512    # 8 f-chunks of 512 (v1 path)
FT = F // P      # 32 f-tiles of 128
DT = D // P      # 8 d-tiles of 128
E = 8
NT = N_TOK
AX = mybir.AxisListType
AF = mybir.ActivationFunctionType
OP = mybir.AluOpType


# ====================================================================
# V4: expert-parallel, host-dispatched, transpose-free.
# ====================================================================


def _chunks(cap):
    """Split cap token columns into <=512-wide PSUM-bank chunks.

    All-but-last chunks are 512 wide; the last carries the remainder so
    the final output copy + DMA on the critical tail is small.
    """
    nch = -(-cap // 512)
    sizes = [512] * (nch - 1) + [cap - 512 * (nch - 1)]
    offs = [0]
    for s in sizes:
        offs.append(offs[-1] + s)
    return nch, sizes, offs


# PE warm-up: dependency-free matmuls bridging the initial weight/x DMA
# so the tensor engine's p-state ramp (cost model: 3us of continuous
# execution) completes before the first real matmul issues.
WARM_N = 64
WARM_COUNT = int(os.environ.get("MOE_WARM", "130"))


def build_nc_v4(cap):
    nch, sizes, offs = _chunks(cap)
    nc = bacc.Bacc(None)
    xT_h = nc.declare_dram_parameter("xT", [D, cap], F16, isOutput=False)
    wi_h = nc.declare_dram_parameter("wi", [FT, P, KD * P], F16, isOutput=False)
    wo_h = nc.declare_dram_parameter("wo", [DT, P, FT * P], F16, isOutput=False)
    yt_h = nc.declare_dram_parameter("yt", [D, cap], F16, isOutput=True)

    with tile.TileContext(nc) as tc:
        with (
            tc.tile_pool(name="persist", bufs=1) as pp,
            tc.tile_pool(name="ps", bufs=8, space="PSUM") as psp,
            tc.tile_pool(name="wi", bufs=3) as wip,
            tc.tile_pool(name="wo", bufs=2) as wop,
            tc.tile_pool(name="yt", bufs=2) as ytp,
        ):
            xT = pp.tile([P, KD, cap], F16, tag="xT")
            hT = pp.tile([P, FT, cap], F16, tag="hT")
            xT_src = xT_h.rearrange("(kd p) t -> p kd t", p=P)

            # Head DMAs, in mm1 consumption order. Each dma_start costs
            # ~650ns of serialized HWDGE descriptor-gen, so keep the count
            # low and the first-needed bytes first: one 2-ft weight batch,
            # then chunk 0 of xT in two halves.
            WIB = 2  # ft tiles per weight DMA batch
            NWB = FT // WIB
            w0 = sizes[0]
            wi_sb0 = wip.tile([P, WIB, KD, P], F16, tag="wi", name="wi0")
            nc.sync.dma_start(
                wi_sb0[:],
                wi_h[0:WIB].rearrange("q p (kd f) -> p q kd f", kd=KD),
            )
            nc.sync.dma_start(xT[:, : KD // 2, :w0], xT_src[:, : KD // 2, :w0])
            nc.sync.dma_start(xT[:, KD // 2 :, :w0], xT_src[:, KD // 2 :, :w0])

            # dependency-free PE warm-up while the head DMAs land
            wsrc = pp.tile([P, WARM_N], F16, tag="wsrc")
            nc.vector.memset(wsrc[:], 0.0)
            wps = psp.tile([P, 512], F32, tag="ps", name="wps")
            for _ in range(WARM_COUNT):
                nc.tensor.matmul(
                    wps[:WARM_N, :WARM_N],
                    lhsT=wsrc[:, :],
                    rhs=wsrc[:, :],
                    start=True,
                    stop=True,
                )

            # mm1: hT[ft, t] = relu(sum_kd W_in[kd, ft].T @ xT[kd, t])
            # Two ft-sweep passes: chunk 0 alone first (so the PE can start
            # after only chunk 0 of xT lands, ~4us earlier), then the
            # remaining chunks together (keeps every pass PE-bound: the
            # remainder chunk alone would be DMA-bound on the wi stream).
            passes = [[0], list(range(1, nch))] if nch > 1 else [[0]]
            for pi, chs in enumerate(passes):
                for wb in range(NWB):
                    if pi == 0 and wb == 0:
                        wi_sb = wi_sb0
                    else:
                        wi_sb = wip.tile(
                            [P, WIB, KD, P], F16, tag="wi", name="wi"
                        )
                        nc.sync.dma_start(
                            wi_sb[:],
                            wi_h[wb * WIB : (wb + 1) * WIB].rearrange(
                                "q p (kd f) -> p q kd f", kd=KD
                            ),
                        )
                    if pi == 0 and wb == 1 and nch > 1:
                        # rest of xT: needed only by pass 1 (~60us away)
                        nc.sync.dma_start(
                            xT[:, :, w0:cap], xT_src[:, :, w0:cap]
                        )
                    for q in range(WIB):
                        ft = wb * WIB + q
                        pss = [
                            psp.tile([P, 512], F32, tag="ps", name=f"ps1_{ch}")
                            for ch in chs
                        ]
                        for kd in range(KD):
                            for ps, ch in zip(pss, chs):
                                o, w = offs[ch], sizes[ch]
                                nc.tensor.matmul(
                                    ps[:, :w],
                                    lhsT=wi_sb[:, q, kd, :],
                                    rhs=xT[:, kd, o : o + w],
                                    start=(kd == 0),
                                    stop=(kd == KD - 1),
                                )
                        for ps, ch in zip(pss, chs):
                            o, w = offs[ch], sizes[ch]
                            nc.scalar.activation(
                                hT[:, ft, o : o + w], ps[:, :w], AF.Relu
                            )

            # mm2: yT[dt, t] = sum_ftc W_out[ftc, dt].T @ hT[ftc, t]
            for dt in range(DT):
                wo_sb = wop.tile([P, FT, P], F16, tag="wo", name="wo")
                nc.sync.dma_start(
                    wo_sb[:], wo_h[dt].rearrange("p (ftc d) -> p ftc d", ftc=FT)
                )
                yt = ytp.tile([P, cap], F16, tag="yt", name="yt")
                split_out = dt == DT - 1
                for ch in range(nch):
                    o, w = offs[ch], sizes[ch]
                    ps = psp.tile([P, 512], F32, tag="ps", name="ps2")
                    for ftc in range(FT):
                        nc.tensor.matmul(
                            ps[:, :w],
                            lhsT=wo_sb[:, ftc, :],
                            rhs=hT[:, ftc, o : o + w],
                            start=(ftc == 0),
                            stop=(ftc == FT - 1),
                        )
                    nc.vector.tensor_copy(yt[:, o : o + w], ps[:, :w])
                    if split_out:
                        nc.sync.dma_start(
                            yt_h[dt * P : (dt + 1) * P, o : o + w],
                            yt[:, o : o + w],
                        )
                if not split_out:
                    nc.sync.dma_start(yt_h[dt * P : (dt + 1) * P, :], yt[:])

    nc.compile()
    return nc


def route_v4(xf, Wr, br):
    """Host router: per-expert token index lists + combine probs."""
    logits = xf @ np.asarray(Wr, np.float32) + np.asarray(
        br, np.float32
    ).reshape(1, E)
    order = np.argsort(-logits, axis=-1, kind="stable")
    top2 = order[:, :2]
    mx = logits.max(axis=-1, keepdims=True)
    ex = np.exp(logits - mx)
    probs = ex / ex.sum(axis=-1, keepdims=True)
    idx_list, p_list = [], []
    for e in range(E):
        sel = np.nonzero((top2 == e).any(axis=1))[0]
        idx_list.append(sel)
        p_list.append(probs[sel, e].astype(np.float32))
    cap = max(16, max(len(s) for s in idx_list))
    cap = -(-cap // 2) * 2
    return idx_list, p_list, cap


def make_in_maps_v4(x, W_in, W_out, idx_list, p_list, cap):
    xf = np.asarray(x, np.float32).reshape(NT, D)
    in_maps = []
    for e in range(E):
        sel = idx_list[e]
        xs = np.zeros((cap, D), np.float32)
        xs[: len(sel)] = xf[sel] * p_list[e][:, None]
        xT = np.ascontiguousarray(xs.T.astype(np.float16))
        wi = np.ascontiguousarray(
            np.asarray(W_in[e], np.float16)
            .reshape(KD, P, FT, P)
            .transpose(2, 1, 0, 3)
        ).reshape(FT, P, KD * P)
        wo = np.ascontiguousarray(
            np.asarray(W_out[e], np.float16)
            .reshape(FT, P, DT, P)
            .transpose(2, 1, 0, 3)
        ).reshape(DT, P, FT * P)
        in_maps.append({"xT": xT, "wi": wi, "wo": wo})
    return in_maps


# ====================================================================
# V1: dense-over-experts data-parallel fallback (handles any biases).
# ====================================================================


def build_nc(cfg):
    """Build the single-core SPMD bass program (dense over experts).

    cfg keys: wdt ('f32r'|'f16') - dtype of expert weights + hT in matmuls;
              has_br/has_bin/has_bout - include bias adds.
    """
    wdt = F32R if cfg["wdt"] == "f32r" else F16
    w_store = F32R if cfg["wdt"] == "f32r" else F16
    has_br = cfg["has_br"]
    has_bin = cfg["has_bin"]
    has_bout = cfg["has_bout"]

    nc = bacc.Bacc(None)
    x_h = nc.declare_dram_parameter("x", [T, D], F32, isOutput=False)
    wr_h = nc.declare_dram_parameter("wr", [D, E], F32, isOutput=False)
    win_h = nc.declare_dram_parameter("w_in", [E, D, F], w_store, isOutput=False)
    wout_h = nc.declare_dram_parameter("w_out", [E, F, D], w_store, isOutput=False)
    br_h = nc.declare_dram_parameter("br", [1, E], F32, isOutput=False) if has_br else None
    bin_h = nc.declare_dram_parameter("b_in", [E, F], F32, isOutput=False) if has_bin else None
    bout_h = nc.declare_dram_parameter("b_out", [E, D], F32, isOutput=False) if has_bout else None
    y_h = nc.declare_dram_parameter("y", [T, D], F32, isOutput=True)

    with tile.TileContext(nc) as tc:
        with (
            tc.tile_pool(name="persist", bufs=1) as pp,
            tc.tile_pool(name="ps", bufs=6, space="PSUM") as psp,
        ):
            ident = pp.tile([P, P], F32, tag="ident")
            from concourse.masks import make_identity
            make_identity(nc, ident[:])

            xT = pp.tile([P, KD, T], F32, tag="xT")          # x transposed, f32
            hT = pp.tile([P, FT, T], w_store, tag="hT")      # h transposed
            xTr = pp.tile([P, KD, T], w_store, tag="xTr", name="xTr")
            wr_sb = pp.tile([P, KD, E], F32, tag="wr")
            disp = pp.tile([P, G * E], F32, tag="disp")      # dispatch mask
            comb = pp.tile([P, G * E], F32, tag="comb")      # combine probs
            yac = [
                pp.tile([P, D], F32, tag=f"y{g}", name=f"yac{g}")
                for g in range(G)
            ]
            ones1 = pp.tile([1, P], F32, tag="ones1")
            if has_bin or has_bout:
                nc.vector.memset(ones1[:], 1.0)
            br_sb = None
            if has_br:
                br_sb = pp.tile([1, E], F32, tag="br")
                nc.sync.dma_start(br_sb[:], br_h[:])

            nc.sync.dma_start(
                wr_sb[:], wr_h[:, :].rearrange("(kd p) e -> p kd e", p=P)
            )

            with tc.tile_pool(name="xload", bufs=2) as xlp:
                for g in range(G):
                    xg = xlp.tile([P, D], F32, tag="xg")
                    nc.sync.dma_start(xg[:], x_h[g * P : (g + 1) * P, :])
                    for kd in range(KD):
                        pst = psp.tile([P, P], F32, tag="ps")
                        nc.tensor.transpose(
                            pst[:], xg[:, kd * P : (kd + 1) * P], ident[:]
                        )
                        nc.vector.tensor_copy(
                            xT[:, kd, g * P : (g + 1) * P], pst[:]
                        )
                        nc.vector.tensor_copy(
                            xTr[:, kd, g * P : (g + 1) * P], pst[:]
                        )

            # router (true fp32 matmul; top-2 must match reference)
            with tc.tile_pool(name="rt", bufs=2) as rtp:
                for g in range(G):
                    psr = psp.tile([P, E], F32, tag="ps")
                    for kd in range(KD):
                        nc.tensor.matmul(
                            psr[:],
                            lhsT=xT[:, kd, g * P : (g + 1) * P],
                            rhs=wr_sb[:, kd, :],
                            start=(kd == 0),
                            stop=(kd == KD - 1 and not has_br),
                        )
                    if has_br:
                        nc.tensor.matmul(
                            psr[:], lhsT=ones1[:, :], rhs=br_sb[:, :],
                            start=False, stop=True,
                        )
                    lg = rtp.tile([P, E], F32, tag="lg")
                    nc.vector.tensor_copy(lg[:], psr[:])
                    mx1 = rtp.tile([P, 1], F32, tag="mx1")
                    nmx = rtp.tile([P, 1], F32, tag="nmx")
                    nc.vector.reduce_max(out=mx1[:], in_=lg[:], axis=AX.X)
                    nc.vector.reduce_max(out=nmx[:], in_=lg[:], axis=AX.X, negate=True)
                    is1 = rtp.tile([P, E], F32, tag="is1")
                    nc.vector.tensor_scalar(
                        out=is1[:], in0=lg[:], scalar1=mx1[:, :1], scalar2=None,
                        op0=OP.is_equal,
                    )
                    lgm = rtp.tile([P, E], F32, tag="lgm")
                    nc.vector.tensor_scalar_mul(is1[:], is1[:], 1e30)
                    nc.vector.tensor_sub(lgm[:], lg[:], is1[:])
                    mx2 = rtp.tile([P, 1], F32, tag="mx2")
                    nc.vector.reduce_max(out=mx2[:], in_=lgm[:], axis=AX.X)
                    dcol = disp[:, g * E : (g + 1) * E]
                    nc.vector.tensor_scalar(
                        out=dcol, in0=lg[:], scalar1=mx2[:, :1], scalar2=None,
                        op0=OP.is_ge,
                    )
                    ex = rtp.tile([P, E], F32, tag="ex")
                    nc.scalar.activation(ex[:], lg[:], AF.Exp, bias=nmx[:, :1])
                    sm = rtp.tile([P, 1], F32, tag="sm")
                    nc.vector.reduce_sum(out=sm[:], in_=ex[:], axis=AX.X)
                    rc = rtp.tile([P, 1], F32, tag="rc")
                    nc.vector.reciprocal(rc[:], sm[:])
                    nc.vector.tensor_scalar_mul(ex[:], ex[:], rc[:, :1])
                    nc.vector.tensor_mul(
                        comb[:, g * E : (g + 1) * E], ex[:], dcol
                    )

            # mm1: h = sum_e mask_e * relu(x@W_in[e] (+ b_in))
            with (
                tc.tile_pool(name="wfe", bufs=2) as wfp,
                tc.tile_pool(name="hf", bufs=2 * G) as hfp,
                tc.tile_pool(name="rtmp", bufs=4) as rtmp,
            ):
                for f in range(FC):
                    hfs = []
                    for e in range(E):
                        wfe = wfp.tile([P, KD, 512], w_store, tag="wfe")
                        nc.sync.dma_start(
                            wfe[:],
                            win_h[e, :, f * 512 : (f + 1) * 512].rearrange(
                                "(kd p) f -> p kd f", p=P
                            ),
                        )
                        if has_bin:
                            bin_sb = wfp.tile([1, 512], F32, tag="bin")
                            nc.sync.dma_start(
                                bin_sb[:],
                                bin_h[e, f * 512 : (f + 1) * 512][None, :],
                            )
                        for g in range(G):
                            ps = psp.tile([P, 512], F32, tag="ps")
                            for kd in range(KD):
                                nc.tensor.matmul(
                                    ps[:],
                                    lhsT=xTr[:, kd, g * P : (g + 1) * P],
                                    rhs=wfe[:, kd, :],
                                    start=(kd == 0),
                                    stop=(kd == KD - 1 and not has_bin),
                                )
                            if has_bin:
                                nc.tensor.matmul(
                                    ps[:],
                                    lhsT=ones1[:, :],
                                    rhs=bin_sb[:, :],
                                    start=False, stop=True,
                                )
                            sc = disp[:, g * E + e : g * E + e + 1]
                            if e == 0:
                                hf = hfp.tile([P, 512], F32, tag="hf")
                                hfs.append(hf)
                                nc.scalar.activation(
                                    hf[:], ps[:], AF.Relu, scale=sc
                                )
                            else:
                                tmp = rtmp.tile([P, 512], F32, tag="rtmp")
                                nc.scalar.activation(
                                    tmp[:], ps[:], AF.Relu, scale=sc
                                )
                                nc.vector.tensor_add(hfs[g][:], hfs[g][:], tmp[:])
                    for g in range(G):
                        for c in range(4):
                            pst = psp.tile([P, P], F32, tag="ps")
                            nc.tensor.transpose(
                                pst[:],
                                hfs[g][:, c * P : (c + 1) * P],
                                ident[:],
                            )
                            nc.vector.tensor_copy(
                                hT[:, f * 4 + c, g * P : (g + 1) * P], pst[:]
                            )

            # mm2: y = sum_e comb_e * (h@W_out[e] (+ b_out))
            ndh = 2 if wdt == F16 else 4
            dw = D // ndh
            with tc.tile_pool(name="wo", bufs=2) as wop:
                for e in range(E):
                    for dh in range(ndh):
                        wo = wop.tile([P, FT, dw], w_store, tag="wo")
                        nc.sync.dma_start(
                            wo[:],
                            wout_h[e, :, dh * dw : (dh + 1) * dw].rearrange(
                                "(ft p) d -> p ft d", p=P
                            ),
                        )
                        if has_bout:
                            bout_sb = wop.tile([1, dw], F32, tag="bout")
                            nc.sync.dma_start(
                                bout_sb[:],
                                bout_h[e, dh * dw : (dh + 1) * dw][None, :],
                            )
                        for g in range(G):
                            ps = psp.tile([P, dw], F32, tag="ps")
                            for ft in range(FT):
                                nc.tensor.matmul(
                                    ps[:],
                                    lhsT=hT[:, ft, g * P : (g + 1) * P],
                                    rhs=wo[:, ft, :],
                                    start=(ft == 0),
                                    stop=(ft == FT - 1 and not has_bout),
                                )
                            if has_bout:
                                nc.tensor.matmul(
                                    ps[:],
                                    lhsT=ones1[:, :],
                                    rhs=bout_sb[:, :],
                                    start=False, stop=True,
                                )
                            cc = comb[:, g * E + e : g * E + e + 1]
                            ysl = yac[g][:, dh * dw : (dh + 1) * dw]
                            if e == 0:
                                nc.vector.tensor_scalar(
                                    out=ysl, in0=ps[:], scalar1=cc,
                                    scalar2=None, op0=OP.mult,
                                )
                            else:
                                tm = wop.tile([P, dw], F32, tag="ytmp")
                                nc.vector.tensor_scalar(
                                    out=tm[:], in0=ps[:], scalar1=cc,
                                    scalar2=None, op0=OP.mult,
                                )
                                nc.vector.tensor_add(ysl, ysl, tm[:])

            for g in range(G):
                nc.sync.dma_start(y_h[g * P : (g + 1) * P, :], yac[g][:])

    nc.compile()
    return nc


_NC_CACHE = {}


def get_nc(cfg_key):
    if cfg_key not in _NC_CACHE:
        cfg = dict(
            wdt=cfg_key[0], has_br=cfg_key[1], has_bin=cfg_key[2],
            has_bout=cfg_key[3],
        )
        _NC_CACHE[cfg_key] = build_nc(cfg)
    return _NC_CACHE[cfg_key]


def get_nc_v4(cap):
    key = ("v4", cap)
    if key not in _NC_CACHE:
        _NC_CACHE[key] = build_nc_v4(cap)
    return _NC_CACHE[key]


WDT_MODE = os.environ.get("MOE_WDT", "f16")


def make_in_maps(x, Wr, br, W_in, b_in, W_out, b_out, wdt_mode):
    xf = np.ascontiguousarray(np.asarray(x, np.float32).reshape(N_TOK, D))
    w_store_np = np.float32 if wdt_mode == "f32r" else np.float16
    win = np.ascontiguousarray(np.asarray(W_in, w_store_np))
    wout = np.ascontiguousarray(np.asarray(W_out, w_store_np))
    wr = np.ascontiguousarray(np.asarray(Wr, np.float32))
    has_br = bool(np.any(np.asarray(br) != 0))
    has_bin = bool(np.any(np.asarray(b_in) != 0))
    has_bout = bool(np.any(np.asarray(b_out) != 0))
    in_maps = []
    for c in range(NCORES):
        m = {
            "x": xf[c * T : (c + 1) * T],
            "wr": wr,
            "w_in": win,
            "w_out": wout,
        }
        if has_br:
            m["br"] = np.asarray(br, np.float32).reshape(1, E)
        if has_bin:
            m["b_in"] = np.asarray(b_in, np.float32)
        if has_bout:
            m["b_out"] = np.asarray(b_out, np.float32)
        in_maps.append(m)
    cfg_key = (wdt_mode, has_br, has_bin, has_bout)
    return cfg_key, in_maps


# v4 = expert-parallel host-dispatched (default); v1 = dense fallback
# (v1 also serves as the general path when b_in/b_out is nonzero)
IMPL = os.environ.get("MOE_IMPL", "v4")


def kernel(x, Wr, br, W_in, b_in, W_out, b_out, top_k):
    assert int(top_k) == 2, "kernel is specialized for top_k=2"
    if IMPL == "v4" and not (np.any(np.asarray(b_in)) or np.any(np.asarray(b_out))):
        xf = np.ascontiguousarray(np.asarray(x, np.float32).reshape(NT, D))
        idx_list, p_list, cap = route_v4(xf, Wr, br)
        in_maps = make_in_maps_v4(x, W_in, W_out, idx_list, p_list, cap)
        nc = get_nc_v4(cap)
        res = run_bass_kernel_spmd(nc, in_maps, list(range(NCORES)))
        y = np.zeros((NT, D), np.float32)
        for e in range(E):
            n = len(idx_list[e])
            ye = np.asarray(res.results[e]["yt"])  # [D, cap] f16
            y[idx_list[e]] += ye[:, :n].T.astype(np.float32)
        return y.reshape(4, 1024, 1024)
    cfg_key, in_maps = make_in_maps(
        x, Wr, br, W_in, b_in, W_out, b_out, WDT_MODE
    )
    nc = get_nc(cfg_key)
    res = run_bass_kernel_spmd(nc, in_maps, list(range(NCORES)))
    y = np.concatenate([res.results[c]["y"] for c in range(NCORES)], axis=0)
    return y.reshape(4, 1024, 1024).astype(np.float32)


# revision 11
# speedup vs baseline: 1.1953x; 1.0023x over previous
"""MoE feed-forward (top-2 of 8 experts) Trainium2 Bass kernel.

Problem: nn_MixtureOfExpertsFeedForward_6734508720763
  x[4,1024,1024] tokens, router Wr[1024,8], experts W_in[8,1024,4096],
  W_out[8,4096,1024], top_k=2.

  ref:  logits = x@Wr + br ; probs = softmax(logits)
        top2 -> dispatch (0/1), combine (prob or 0)
        h = sum_e dispatch[n,e] * relu(x @ W_in[e] + b_in[e])
        y = sum_e combine[n,e]  * (h @ W_out[e] + b_out[e])

V4 strategy (expert parallelism, host-side all-to-all dispatch):
  Core e owns expert e. The host computes the (tiny, 67 MFLOP) router,
  gathers each expert's routed tokens, pre-scales each token row by its
  combine prob p (valid since p>0: p*relu(z) == relu(p*z) and the output
  Linear is linear), pads every expert to a common CAP so the SPMD
  program is shape-identical, and scatter-adds the per-expert outputs.

  The device program per core is a dense relu(x @ W_in) @ W_out with the
  WEIGHTS as the stationary matmul operand and the tokens as the moving
  (free) axis:
    mm1:  hT[ftile, t] += W_in[kd, ftile].T @ xT[kd, t]   (accum over kd)
    mm2:  yT[dtile, t] += W_out[ftc, dtile].T @ hT[ftc, t] (accum over ftc)
  so mm1's output is already transposed for mm2 -> ZERO PE transposes and
  the PE stream is nothing but back-to-back fp16 matmuls. x is gathered /
  transposed / fp16-cast on the host; weights are host-pretiled so every
  DMA row is >=2KB contiguous.

V1 fallback (dense over experts, data parallel) retained for nonzero
b_in/b_out inputs.
"""

import os
import sys

import numpy as np

sys.path.insert(0, "/opt/trn_rl_repo")

import concourse.bacc as bacc
import concourse.bass as bass
import concourse.mybir as mybir
import concourse.tile as tile
from concourse.bass_utils import run_bass_kernel_spmd

F32 = mybir.dt.float32
F32R = mybir.dt.float32r
F16 = mybir.dt.float16

P = 128          # partitions
NCORES = 8
N_TOK = 4096     # total tokens (4*1024)
T = N_TOK // NCORES   # tokens per core = 512 (v1 path)
G = T // P       # token groups per core = 4 (v1 path)
D = 1024
KD = D // P      # 8 contraction chunks for D
F = 4096
FC = F // 512    # 8 f-chunks of 512 (v1 path)
FT = F // P      # 32 f-tiles of 128
DT = D // P      # 8 d-tiles of 128
E = 8
NT = N_TOK
AX = mybir.AxisListType
AF = mybir.ActivationFunctionType
OP = mybir.AluOpType


# ====================================================================
# V4: expert-parallel, host-dispatched, transpose-free.
# ====================================================================


def _chunks(cap):
    """Split cap token columns into <=512-wide PSUM-bank chunks.

    All-but-last chunks are 512 wide; the last carries the remainder so
    the final output copy + DMA on the critical tail is small.
    """
    nch = -(-cap // 512)
    sizes = [512] * (nch - 1) + [cap - 512 * (nch - 1)]
    offs = [0]
    for s in sizes:
        offs.append(offs[-1] + s)
    return nch, sizes, offs


# PE warm-up: dependency-free matmuls bridging the initial weight/x DMA
# so the tensor engine's p-state ramp (cost model: 3us of continuous
# execution) completes before the first real matmul issues.
WARM_N = 64
WARM_COUNT = int(os.environ.get("MOE_WARM", "130"))


def build_nc_v4(cap):
    nch, sizes, offs = _chunks(cap)
    nc = bacc.Bacc(None)
    xT_h = nc.declare_dram_parameter("xT", [D, cap], F16, isOutput=False)
    wi_h = nc.declare_dram_parameter("wi", [FT, P, KD * P], F16, isOutput=False)
    wo_h = nc.declare_dram_parameter("wo", [DT, P, FT * P], F16, isOutput=False)
    yt_h = nc.declare_dram_parameter("yt", [D, cap], F16, isOutput=True)

    with tile.TileContext(nc) as tc:
        with (
            tc.tile_pool(name="persist", bufs=1) as pp,
            tc.tile_pool(name="ps", bufs=8, space="PSUM") as psp,
            tc.tile_pool(name="wi", bufs=3) as wip,
            tc.tile_pool(name="wo", bufs=2) as wop,
            tc.tile_pool(name="yt", bufs=2) as ytp,
        ):
            xT = pp.tile([P, KD, cap], F16, tag="xT")
            hT = pp.tile([P, FT, cap], F16, tag="hT")
            xT_src = xT_h.rearrange("(kd p) t -> p kd t", p=P)

            # Head DMAs, in mm1 consumption order. Each dma_start costs
            # ~650ns of serialized HWDGE descriptor-gen, so keep the count
            # low and the first-needed bytes first: one 2-ft weight batch,
            # then chunk 0 of xT in two halves.
            WIB = 2  # ft tiles per weight DMA batch
            NWB = FT // WIB
            w0 = sizes[0]
            wi_sb0 = wip.tile([P, WIB, KD, P], F16, tag="wi", name="wi0")
            nc.sync.dma_start(
                wi_sb0[:],
                wi_h[0:WIB].rearrange("q p (kd f) -> p q kd f", kd=KD),
            )
            nc.sync.dma_start(xT[:, : KD // 2, :w0], xT_src[:, : KD // 2, :w0])
            nc.sync.dma_start(xT[:, KD // 2 :, :w0], xT_src[:, KD // 2 :, :w0])

            # dependency-free PE warm-up while the head DMAs land
            wsrc = pp.tile([P, WARM_N], F16, tag="wsrc")
            nc.vector.memset(wsrc[:], 0.0)
            wps = psp.tile([P, 512], F32, tag="ps", name="wps")
            for _ in range(WARM_COUNT):
                nc.tensor.matmul(
                    wps[:WARM_N, :WARM_N],
                    lhsT=wsrc[:, :],
                    rhs=wsrc[:, :],
                    start=True,
                    stop=True,
                )

            # mm1: hT[ft, t] = relu(sum_kd W_in[kd, ft].T @ xT[kd, t])
            # Two ft-sweep passes: chunk 0 alone first (so the PE can start
            # after only chunk 0 of xT lands, ~4us earlier), then the
            # remaining chunks together (keeps every pass PE-bound: the
            # remainder chunk alone would be DMA-bound on the wi stream).
            passes = [[0], list(range(1, nch))] if nch > 1 else [[0]]
            for pi, chs in enumerate(passes):
                for wb in range(NWB):
                    if pi == 0 and wb == 0:
                        wi_sb = wi_sb0
                    else:
                        wi_sb = wip.tile(
                            [P, WIB, KD, P], F16, tag="wi", name="wi"
                        )
                        nc.sync.dma_start(
                            wi_sb[:],
                            wi_h[wb * WIB : (wb + 1) * WIB].rearrange(
                                "q p (kd f) -> p q kd f", kd=KD
                            ),
                        )
                    if pi == 0 and wb == 4 and nch > 1:
                        # rest of xT: needed only by pass 1 (~60us away)
                        nc.sync.dma_start(
                            xT[:, :, w0:cap], xT_src[:, :, w0:cap]
                        )
                    for q in range(WIB):
                        ft = wb * WIB + q
                        pss = [
                            psp.tile([P, 512], F32, tag="ps", name=f"ps1_{ch}")
                            for ch in chs
                        ]
                        for kd in range(KD):
                            for ps, ch in zip(pss, chs):
                                o, w = offs[ch], sizes[ch]
                                nc.tensor.matmul(
                                    ps[:, :w],
                                    lhsT=wi_sb[:, q, kd, :],
                                    rhs=xT[:, kd, o : o + w],
                                    start=(kd == 0),
                                    stop=(kd == KD - 1),
                                )
                        for ps, ch in zip(pss, chs):
                            o, w = offs[ch], sizes[ch]
                            nc.scalar.activation(
                                hT[:, ft, o : o + w], ps[:, :w], AF.Relu
                            )

            # mm2: yT[dt, t] = sum_ftc W_out[ftc, dt].T @ hT[ftc, t]
            for dt in range(DT):
                wo_sb = wop.tile([P, FT, P], F16, tag="wo", name="wo")
                nc.sync.dma_start(
                    wo_sb[:], wo_h[dt].rearrange("p (ftc d) -> p ftc d", ftc=FT)
                )
                yt = ytp.tile([P, cap], F16, tag="yt", name="yt")
                split_out = dt == DT - 1
                for ch in range(nch):
                    o, w = offs[ch], sizes[ch]
                    ps = psp.tile([P, 512], F32, tag="ps", name="ps2")
                    for ftc in range(FT):
                        nc.tensor.matmul(
                            ps[:, :w],
                            lhsT=wo_sb[:, ftc, :],
                            rhs=hT[:, ftc, o : o + w],
                            start=(ftc == 0),
                            stop=(ftc == FT - 1),
                        )
                    nc.vector.tensor_copy(yt[:, o : o + w], ps[:, :w])
                    if split_out:
                        nc.sync.dma_start(
                            yt_h[dt * P : (dt + 1) * P, o : o + w],
                            yt[:, o : o + w],
                        )
                if not split_out:
                    nc.sync.dma_start(yt_h[dt * P : (dt + 1) * P, :], yt[:])

    nc.compile()
    return nc


def route_v4(xf, Wr, br):
    """Host router: per-expert token index lists + combine probs."""
    logits = xf @ np.asarray(Wr, np.float32) + np.asarray(
        br, np.float32
    ).reshape(1, E)
    order = np.argsort(-logits, axis=-1, kind="stable")
    top2 = order[:, :2]
    mx = logits.max(axis=-1, keepdims=True)
    ex = np.exp(logits - mx)
    probs = ex / ex.sum(axis=-1, keepdims=True)
    idx_list, p_list = [], []
    for e in range(E):
        sel = np.nonzero((top2 == e).any(axis=1))[0]
        idx_list.append(sel)
        p_list.append(probs[sel, e].astype(np.float32))
    cap = max(16, max(len(s) for s in idx_list))
    cap = -(-cap // 2) * 2
    return idx_list, p_list, cap


def make_in_maps_v4(x, W_in, W_out, idx_list, p_list, cap):
    xf = np.asarray(x, np.float32).reshape(NT, D)
    in_maps = []
    for e in range(E):
        sel = idx_list[e]
        xs = np.zeros((cap, D), np.float32)
        xs[: len(sel)] = xf[sel] * p_list[e][:, None]
        xT = np.ascontiguousarray(xs.T.astype(np.float16))
        wi = np.ascontiguousarray(
            np.asarray(W_in[e], np.float16)
            .reshape(KD, P, FT, P)
            .transpose(2, 1, 0, 3)
        ).reshape(FT, P, KD * P)
        wo = np.ascontiguousarray(
            np.asarray(W_out[e], np.float16)
            .reshape(FT, P, DT, P)
            .transpose(2, 1, 0, 3)
        ).reshape(DT, P, FT * P)
        in_maps.append({"xT": xT, "wi": wi, "wo": wo})
    return in_maps


# ====================================================================
# V1: dense-over-experts data-parallel fallback (handles any biases).
# ====================================================================


def build_nc(cfg):
    """Build the single-core SPMD bass program (dense over experts).

    cfg keys: wdt ('f32r'|'f16') - dtype of expert weights + hT in matmuls;
              has_br/has_bin/has_bout - include bias adds.
    """
    wdt = F32R if cfg["wdt"] == "f32r" else F16
    w_store = F32R if cfg["wdt"] == "f32r" else F16
    has_br = cfg["has_br"]
    has_bin = cfg["has_bin"]
    has_bout = cfg["has_bout"]

    nc = bacc.Bacc(None)
    x_h = nc.declare_dram_parameter("x", [T, D], F32, isOutput=False)
    wr_h = nc.declare_dram_parameter("wr", [D, E], F32, isOutput=False)
    win_h = nc.declare_dram_parameter("w_in", [E, D, F], w_store, isOutput=False)
    wout_h = nc.declare_dram_parameter("w_out", [E, F, D], w_store, isOutput=False)
    br_h = nc.declare_dram_parameter("br", [1, E], F32, isOutput=False) if has_br else None
    bin_h = nc.declare_dram_parameter("b_in", [E, F], F32, isOutput=False) if has_bin else None
    bout_h = nc.declare_dram_parameter("b_out", [E, D], F32, isOutput=False) if has_bout else None
    y_h = nc.declare_dram_parameter("y", [T, D], F32, isOutput=True)

    with tile.TileContext(nc) as tc:
        with (
            tc.tile_pool(name="persist", bufs=1) as pp,
            tc.tile_pool(name="ps", bufs=6, space="PSUM") as psp,
        ):
            ident = pp.tile([P, P], F32, tag="ident")
            from concourse.masks import make_identity
            make_identity(nc, ident[:])

            xT = pp.tile([P, KD, T], F32, tag="xT")          # x transposed, f32
            hT = pp.tile([P, FT, T], w_store, tag="hT")      # h transposed
            xTr = pp.tile([P, KD, T], w_store, tag="xTr", name="xTr")
            wr_sb = pp.tile([P, KD, E], F32, tag="wr")
            disp = pp.tile([P, G * E], F32, tag="disp")      # dispatch mask
            comb = pp.tile([P, G * E], F32, tag="comb")      # combine probs
            yac = [
                pp.tile([P, D], F32, tag=f"y{g}", name=f"yac{g}")
                for g in range(G)
            ]
            ones1 = pp.tile([1, P], F32, tag="ones1")
            if has_bin or has_bout:
                nc.vector.memset(ones1[:], 1.0)
            br_sb = None
            if has_br:
                br_sb = pp.tile([1, E], F32, tag="br")
                nc.sync.dma_start(br_sb[:], br_h[:])

            nc.sync.dma_start(
                wr_sb[:], wr_h[:, :].rearrange("(kd p) e -> p kd e", p=P)
            )

            with tc.tile_pool(name="xload", bufs=2) as xlp:
                for g in range(G):
                    xg = xlp.tile([P, D], F32, tag="xg")
                    nc.sync.dma_start(xg[:], x_h[g * P : (g + 1) * P, :])
                    for kd in range(KD):
                        pst = psp.tile([P, P], F32, tag="ps")
                        nc.tensor.transpose(
                            pst[:], xg[:, kd * P : (kd + 1) * P], ident[:]
                        )
                        nc.vector.tensor_copy(
                            xT[:, kd, g * P : (g + 1) * P], pst[:]
                        )
                        nc.vector.tensor_copy(
                            xTr[:, kd, g * P : (g + 1) * P], pst[:]
                        )

            # router (true fp32 matmul; top-2 must match reference)
            with tc.tile_pool(name="rt", bufs=2) as rtp:
                for g in range(G):
                    psr = psp.tile([P, E], F32, tag="ps")
                    for kd in range(KD):
                        nc.tensor.matmul(
                            psr[:],
                            lhsT=xT[:, kd, g * P : (g + 1) * P],
                            rhs=wr_sb[:, kd, :],
                            start=(kd == 0),
                            stop=(kd == KD - 1 and not has_br),
                        )
                    if has_br:
                        nc.tensor.matmul(
                            psr[:], lhsT=ones1[:, :], rhs=br_sb[:, :],
                            start=False, stop=True,
                        )
                    lg = rtp.tile([P, E], F32, tag="lg")
                    nc.vector.tensor_copy(lg[:], psr[:])
                    mx1 = rtp.tile([P, 1], F32, tag="mx1")
                    nmx = rtp.tile([P, 1], F32, tag="nmx")
                    nc.vector.reduce_max(out=mx1[:], in_=lg[:], axis=AX.X)
                    nc.vector.reduce_max(out=nmx[:], in_=lg[:], axis=AX.X, negate=True)
                    is1 = rtp.tile([P, E], F32, tag="is1")
                    nc.vector.tensor_scalar(
                        out=is1[:], in0=lg[:], scalar1=mx1[:, :1], scalar2=None,
                        op0=OP.is_equal,
                    )
                    lgm = rtp.tile([P, E], F32, tag="lgm")
                    nc.vector.tensor_scalar_mul(is1[:], is1[:], 1e30)
                    nc.vector.tensor_sub(lgm[:], lg[:], is1[:])
                    mx2 = rtp.tile([P, 1], F32, tag="mx2")
                    nc.vector.reduce_max(out=mx2[:], in_=lgm[:], axis=AX.X)
                    dcol = disp[:, g * E : (g + 1) * E]
                    nc.vector.tensor_scalar(
                        out=dcol, in0=lg[:], scalar1=mx2[:, :1], scalar2=None,
                        op0=OP.is_ge,
                    )
                    ex = rtp.tile([P, E], F32, tag="ex")
                    nc.scalar.activation(ex[:], lg[:], AF.Exp, bias=nmx[:, :1])
                    sm = rtp.tile([P, 1], F32, tag="sm")
                    nc.vector.reduce_sum(out=sm[:], in_=ex[:], axis=AX.X)
                    rc = rtp.tile([P, 1], F32, tag="rc")
                    nc.vector.reciprocal(rc[:], sm[:])
                    nc.vector.tensor_scalar_mul(ex[:], ex[:], rc[:, :1])
                    nc.vector.tensor_mul(
                        comb[:, g * E : (g + 1) * E], ex[:], dcol
                    )

            # mm1: h = sum_e mask_e * relu(x@W_in[e] (+ b_in))
            with (
                tc.tile_pool(name="wfe", bufs=2) as wfp,
                tc.tile_pool(name="hf", bufs=2 * G) as hfp,
                tc.tile_pool(name="rtmp", bufs=4) as rtmp,
            ):
                for f in range(FC):
                    hfs = []
                    for e in range(E):
                        wfe = wfp.tile([P, KD, 512], w_store, tag="wfe")
                        nc.sync.dma_start(
                            wfe[:],
                            win_h[e, :, f * 512 : (f + 1) * 512].rearrange(
                                "(kd p) f -> p kd f", p=P
                            ),
                        )
                        if has_bin:
                            bin_sb = wfp.tile([1, 512], F32, tag="bin")
                            nc.sync.dma_start(
                                bin_sb[:],
                                bin_h[e, f * 512 : (f + 1) * 512][None, :],
                            )
                        for g in range(G):
                            ps = psp.tile([P, 512], F32, tag="ps")
                            for kd in range(KD):
                                nc.tensor.matmul(
                                    ps[:],
                                    lhsT=xTr[:, kd, g * P : (g + 1) * P],
                                    rhs=wfe[:, kd, :],
                                    start=(kd == 0),
                                    stop=(kd == KD - 1 and not has_bin),
                                )
                            if has_bin:
                                nc.tensor.matmul(
                                    ps[:],
                                    lhsT=ones1[:, :],
                                    rhs=bin_sb[:, :],
                                    start=False, stop=True,
                                )
                            sc = disp[:, g * E + e : g * E + e + 1]
                            if e == 0:
                                hf = hfp.tile([P, 512], F32, tag="hf")
                                hfs.append(hf)
                                nc.scalar.activation(
                                    hf[:], ps[:], AF.Relu, scale=sc
                                )
                            else:
                                tmp = rtmp.tile([P, 512], F32, tag="rtmp")
                                nc.scalar.activation(
                                    tmp[:], ps[:], AF.Relu, scale=sc
                                )
                                nc.vector.tensor_add(hfs[g][:], hfs[g][:], tmp[:])
                    for g in range(G):
                        for c in range(4):
                            pst = psp.tile([P, P], F32, tag="ps")
                            nc.tensor.transpose(
                                pst[:],
                                hfs[g][:, c * P : (c + 1) * P],
                                ident[:],
                            )
                            nc.vector.tensor_copy(
                                hT[:, f * 4 + c, g * P : (g + 1) * P], pst[:]
                            )

            # mm2: y = sum_e comb_e * (h@W_out[e] (+ b_out))
            ndh = 2 if wdt == F16 else 4
            dw = D // ndh
            with tc.tile_pool(name="wo", bufs=2) as wop:
                for e in range(E):
                    for dh in range(ndh):
                        wo = wop.tile([P, FT, dw], w_store, tag="wo")
                        nc.sync.dma_start(
                            wo[:],
                            wout_h[e, :, dh * dw : (dh + 1) * dw].rearrange(
                                "(ft p) d -> p ft d", p=P
                            ),
                        )
                        if has_bout:
                            bout_sb = wop.tile([1, dw], F32, tag="bout")
                            nc.sync.dma_start(
                                bout_sb[:],
                                bout_h[e, dh * dw : (dh + 1) * dw][None, :],
                            )
                        for g in range(G):
                            ps = psp.tile([P, dw], F32, tag="ps")
                            for ft in range(FT):
                                nc.tensor.matmul(
                                    ps[:],
                                    lhsT=hT[:, ft, g * P : (g + 1) * P],
                                    rhs=wo[:, ft, :],
                                    start=(ft == 0),
                                    stop=(ft == FT - 1 and not has_bout),
                                )
                            if has_bout:
                                nc.tensor.matmul(
                                    ps[:],
                                    lhsT=ones1[:, :],
                                    rhs=bout_sb[:, :],
                                    start=False, stop=True,
                                )
                            cc = comb[:, g * E + e : g * E + e + 1]
                            ysl = yac[g][:, dh * dw : (dh + 1) * dw]
                            if e == 0:
                                nc.vector.tensor_scalar(
                                    out=ysl, in0=ps[:], scalar1=cc,
                                    scalar2=None, op0=OP.mult,
                                )
                            else:
                                tm = wop.tile([P, dw], F32, tag="ytmp")
                                nc.vector.tensor_scalar(
                                    out=tm[:], in0=ps[:], scalar1=cc,
                                    scalar2=None, op0=OP.mult,
                                )
                                nc.vector.tensor_add(ysl, ysl, tm[:])

            for g in range(G):
                nc.sync.dma_start(y_h[g * P : (g + 1) * P, :], yac[g][:])

    nc.compile()
    return nc


_NC_CACHE = {}


def get_nc(cfg_key):
    if cfg_key not in _NC_CACHE:
        cfg = dict(
            wdt=cfg_key[0], has_br=cfg_key[1], has_bin=cfg_key[2],
            has_bout=cfg_key[3],
        )
        _NC_CACHE[cfg_key] = build_nc(cfg)
    return _NC_CACHE[cfg_key]


def get_nc_v4(cap):
    key = ("v4", cap)
    if key not in _NC_CACHE:
        _NC_CACHE[key] = build_nc_v4(cap)
    return _NC_CACHE[key]


WDT_MODE = os.environ.get("MOE_WDT", "f16")


def make_in_maps(x, Wr, br, W_in, b_in, W_out, b_out, wdt_mode):
    xf = np.ascontiguousarray(np.asarray(x, np.float32).reshape(N_TOK, D))
    w_store_np = np.float32 if wdt_mode == "f32r" else np.float16
    win = np.ascontiguousarray(np.asarray(W_in, w_store_np))
    wout = np.ascontiguousarray(np.asarray(W_out, w_store_np))
    wr = np.ascontiguousarray(np.asarray(Wr, np.float32))
    has_br = bool(np.any(np.asarray(br) != 0))
    has_bin = bool(np.any(np.asarray(b_in) != 0))
    has_bout = bool(np.any(np.asarray(b_out) != 0))
    in_maps = []
    for c in range(NCORES):
        m = {
            "x": xf[c * T : (c + 1) * T],
            "wr": wr,
            "w_in": win,
            "w_out": wout,
        }
        if has_br:
            m["br"] = np.asarray(br, np.float32).reshape(1, E)
        if has_bin:
            m["b_in"] = np.asarray(b_in, np.float32)
        if has_bout:
            m["b_out"] = np.asarray(b_out, np.float32)
        in_maps.append(m)
    cfg_key = (wdt_mode, has_br, has_bin, has_bout)
    return cfg_key, in_maps


# v4 = expert-parallel host-dispatched (default); v1 = dense fallback
# (v1 also serves as the general path when b_in/b_out is nonzero)
IMPL = os.environ.get("MOE_IMPL", "v4")


def kernel(x, Wr, br, W_in, b_in, W_out, b_out, top_k):
    assert int(top_k) == 2, "kernel is specialized for top_k=2"
    if IMPL == "v4" and not (np.any(np.asarray(b_in)) or np.any(np.asarray(b_out))):
        xf = np.ascontiguousarray(np.asarray(x, np.float32).reshape(NT, D))
        idx_list, p_list, cap = route_v4(xf, Wr, br)
        in_maps = make_in_maps_v4(x, W_in, W_out, idx_list, p_list, cap)
        nc = get_nc_v4(cap)
        res = run_bass_kernel_spmd(nc, in_maps, list(range(NCORES)))
        y = np.zeros((NT, D), np.float32)
        for e in range(E):
            n = len(idx_list[e])
            ye = np.asarray(res.results[e]["yt"])  # [D, cap] f16
            y[idx_list[e]] += ye[:, :n].T.astype(np.float32)
        return y.reshape(4, 1024, 1024)
    cfg_key, in_maps = make_in_maps(
        x, Wr, br, W_in, b_in, W_out, b_out, WDT_MODE
    )
    nc = get_nc(cfg_key)
    res = run_bass_kernel_spmd(nc, in_maps, list(range(NCORES)))
    y = np.concatenate([res.results[c]["y"] for c in range(NCORES)], axis=0)
    return y.reshape(4, 1024, 1024).astype(np.float32)
